# revision 20
# baseline (speedup 1.0000x reference)
"""MoE routed conv for Trainium2, 8-core SPMD — 1D Winograd F(2,3).

Math: each batch image selects one expert (argmax of scores); output equals
a 3x3 pad-1 conv with the selected expert's filter. We compute only that
expert's conv (5x less work), data-parallel 4 images per core.

Structure per image (PE window ~7.84us, two row-halves of 28):
- PE: per half, j=0..3 Winograd taps x 3 kh x 2 row-chunks of 14 -> 24
  matmuls accumulating into j-pair PSUM mega-tiles ([OC,2,2,512]: (j,chunk)
  in its own bank so start=True zeroing stays chunk-local).
- Act drains each completed j-pair to SBUF bf16 (a01 = [OC,2,2,392]) --
  2 big drains per half; PSUM tiles release with ~2us of pipeline cover.
- Combines (all SBUF, contiguous 784-elem halves): oe = (a0+a1)+a2,
  oo = (a1-a2)-a3.  GpSimd: oe1 = a0+a1; DVE: oe, t12, oo.
- Input: images 0,1 arrive host-Winograd-transformed (D planes, j-major
  streaming so the PE starts after two DMAs); images 2,3 arrive as raw
  de-interleaved even/odd padded columns and DVE computes their D planes
  (4 contiguous bf16 TTs at 2x) one image ahead.
Output is stored as parity planes [OC,2,H,28]; host interleaves columns.
"""
import numpy as np

B, C, H, W = 32, 128, 56, 56
E, OC = 5, 128
NCORES = 8
IPC = B // NCORES          # images per core
NHOST = 2                  # images with host-side input transform
WE = 29                    # de-interleaved padded width (per parity)
G = W // 2                 # 28 column pairs
CH2 = 14                   # output rows per PSUM chunk
HH = 28                    # rows per half

_program = None


def _build_program():
    import concourse.bacc as bacc
    import concourse.tile as tile
    from concourse.tile import add_dep_helper
    from concourse import mybir

    dt = mybir.dt
    idt = dt.bfloat16
    nc = bacc.Bacc("TRN2", target_bir_lowering=False, debug=False)
    # images 0..NHOST-1: host-side Winograd input transform, [img, ci, j, h, g]
    d0_d = nc.dram_tensor("d0", [NHOST, C, 4, H, G], idt,
                          kind="ExternalInput").ap()
    # images NHOST..: de-interleaved padded columns [img-NHOST, ci, parity, h, 29]
    x_d = nc.dram_tensor("x", [IPC - NHOST, C, 2, H, WE], idt,
                         kind="ExternalInput").ap()
    w_d = nc.dram_tensor("w", [IPC, C, 12, OC], idt, kind="ExternalInput").ap()
    # o: parity planes, [img, co, parity(e/o), h, g]
    o_d = nc.dram_tensor("o", [IPC, OC, 2, H, G], idt, kind="ExternalOutput").ap()

    Copy = mybir.ActivationFunctionType.Copy

    with tile.TileContext(nc) as tc:
        with (
            tc.tile_pool(name="xp", bufs=1) as xp,
            tc.tile_pool(name="dp", bufs=1) as dp,
            tc.tile_pool(name="wpool", bufs=1) as wpool,
            tc.tile_pool(name="opool", bufs=1) as opool,
            tc.tile_pool(name="tpool", bufs=16) as tpool,
            tc.tile_pool(name="ps", bufs=4, space="PSUM") as psp,
        ):
            xts = [xp.tile([C, 2, H, WE], idt, name=f"xt{i}") for i in range(2)]
            dts = [dp.tile([C, 4, H, G], idt, name=f"dt{i}") for i in range(2)]
            wts = [wpool.tile([C, 12, OC], idt, name=f"wt{i}") for i in range(IPC)]
            ots = [opool.tile([OC, 2, H, G], idt, name=f"ot{i}") for i in range(2)]

            def in_transform(img):
                """D0..D3 for one raw image on DVE; contiguous bf16 SBUF ops."""
                xt = xts[img % 2]
                D = dts[img % 2]
                xe0 = xt[:, 0, :, 0:G]
                xe1 = xt[:, 0, :, 1:G + 1]
                xo0 = xt[:, 1, :, 0:G]
                xo1 = xt[:, 1, :, 1:G + 1]
                nc.vector.tensor_sub(D[:, 0], xe0, xe1)   # d0 - d2
                nc.vector.tensor_add(D[:, 1], xo0, xe1)   # d1 + d2
                nc.vector.tensor_sub(D[:, 2], xe1, xo0)   # d2 - d1
                nc.vector.tensor_sub(D[:, 3], xo0, xo1)   # d1 - d3

            # head: w0's j0 taps alone on the scalar queue; image-0 D planes
            # j-major (j0 row-split) on the sync queue so the first matmul
            # gates on only ~300KB of DMA.
            nc.scalar.dma_start(out=wts[0][:, 0:3], in_=w_d[0, :, 0:3])
            nc.sync.dma_start(out=dts[0][:, 0, 0:HH + 1], in_=d0_d[0, :, 0, 0:HH + 1])
            nc.sync.dma_start(out=dts[0][:, 0, HH + 1:H], in_=d0_d[0, :, 0, HH + 1:H])
            nc.sync.dma_start(out=dts[0][:, 1, 0:HH + 1], in_=d0_d[0, :, 1, 0:HH + 1])
            nc.scalar.dma_start(out=wts[0][:, 3:12], in_=w_d[0, :, 3:12])
            nc.sync.dma_start(out=dts[0][:, 1, HH + 1:H], in_=d0_d[0, :, 1, HH + 1:H])
            for j in range(2, 4):
                nc.sync.dma_start(out=dts[0][:, j], in_=d0_d[0, :, j])
            # image 1 (host-transformed) + w1 follow
            nc.scalar.dma_start(out=wts[1][:], in_=w_d[1])
            for j in range(4):
                nc.sync.dma_start(out=dts[1][:, j], in_=d0_d[1, :, j])

            anchor = None
            for img in range(IPC):
                dtile = dts[img % 2]
                wt = wts[img]
                ot = ots[img % 2]
                # next raw image's input transform in this image's DVE window
                if NHOST <= img + 1 < IPC:
                    in_transform(img + 1)

                for hf in range(2):
                    r0h = hf * HH
                    last = img == IPC - 1 and hf == 1
                    # a[j] holds drained planes (2chunk, 392)
                    aj = [tpool.tile([OC, 2, 392], idt,
                                     name=f"a{j}_{img}_{hf}", tag="tm")
                          for j in range(4)]
                    oe1 = tpool.tile([OC, HH, G], idt,
                                     name=f"oe1_{img}_{hf}", tag="tm")
                    t12 = tpool.tile([OC, HH, G], idt,
                                     name=f"t12_{img}_{hf}", tag="tm")
                    a0, a1, a2, a3 = (aj[j][:] for j in range(4))
                    hsl = slice(r0h, r0h + HH)
                    for j in range(4):
                        ps = psp.tile([OC, 2, 512], dt.float32,
                                      name=f"ps{img}_{hf}_{j}", tag="ps")
                        if last and j == 3:
                            sweep = [(kh, c2) for c2 in range(2)
                                     for kh in range(3)]
                        else:
                            sweep = [(kh, c2) for kh in range(3)
                                     for c2 in range(2)]
                        for (kh, c2) in sweep:
                            r0 = r0h + c2 * CH2
                            hs = max(r0, 1 - kh)
                            he = min(r0 + CH2, H + 1 - kh)
                            rhs = dtile[:, j, hs + kh - 1 : he + kh - 1, :]
                            out = ps[:, c2, (hs - r0) * G : (he - r0) * G]
                            mm = nc.tensor.matmul(out, wt[:, j * 3 + kh, :],
                                                  rhs, start=(kh == 0),
                                                  stop=(kh == 2))
                            if img == 0 and hf == 0 and j == 0 \
                                    and kh == 0 and c2 == 1:
                                anchor = mm
                        # drain the completed j-plane (releasing its PSUM tile
                        # with a ~3.9us next-use cover) + combines as ready
                        if last and j == 3:
                            # tail: j3 per-chunk for early odd-plane stores
                            for c2 in range(2):
                                cs = slice(r0h + c2 * CH2,
                                           r0h + (c2 + 1) * CH2)
                                nc.scalar.activation(aj[3][:, c2],
                                                     ps[:, c2, 0:392], Copy)
                                nc.vector.tensor_sub(
                                    ot[:, 1, cs, :],
                                    t12[:, c2 * CH2:(c2 + 1) * CH2, :],
                                    aj[3][:, c2])
                                qs = nc.gpsimd if c2 == 0 else nc.scalar
                                qs.dma_start(out=o_d[img, :, 1, cs, :],
                                             in_=ot[:, 1, cs, :])
                        else:
                            nc.scalar.activation(aj[j][:], ps[:, :, 0:392],
                                                 Copy)
                        if j == 1:
                            nc.vector.tensor_add(oe1[:], a0, a1)
                        elif j == 2:
                            nc.vector.tensor_add(ot[:, 0, hsl, :], oe1[:], a2)
                            nc.vector.tensor_sub(t12[:], a1, a2)
                            if last:
                                # even plane of this half is complete: store it
                                nc.sync.dma_start(out=o_d[img, :, 0, hsl, :],
                                                  in_=ot[:, 0, hsl, :])
                        elif j == 3 and not last:
                            nc.vector.tensor_sub(ot[:, 1, hsl, :], t12[:], a3)
                            if img == IPC - 1:
                                nc.gpsimd.dma_start(
                                    out=o_d[img, :, :, hsl, :],
                                    in_=ot[:, :, hsl, :])
                if img < IPC - 1:
                    nc.gpsimd.dma_start(out=o_d[img], in_=ot[:])
                # prefetch raw images two ahead on the scalar queue, delayed
                # past the head-critical DMAs
                pf = img + 2
                if NHOST <= pf < IPC:
                    xt = xts[pf % 2]
                    loads = [nc.scalar.dma_start(out=xt[:], in_=x_d[pf - NHOST]),
                             nc.scalar.dma_start(out=wts[pf][:], in_=w_d[pf])]
                    for ld in loads:
                        add_dep_helper(ld.ins, anchor.ins, sync=True,
                                       reason="delay prefetch past head")
    nc.compile()
    return nc


def _get_program():
    global _program
    if _program is None:
        _program = _build_program()
    return _program


def kernel(x: np.ndarray, scores: np.ndarray, weight: np.ndarray,
           **run_kwargs) -> np.ndarray:
    import ml_dtypes
    from concourse.bass_utils import run_bass_kernel_spmd

    x = np.asarray(x, dtype=np.float32)
    scores = np.asarray(scores, dtype=np.float32)
    weight = np.asarray(weight, dtype=np.float32)

    expert = np.argmax(scores, axis=1)                       # [B]
    w_sel = weight.reshape(E, OC, C, 3, 3)[expert]           # [B, co, ci, kh, kw]
    # Winograd weight transform G.w per kh: [B, co, ci, kh, j]
    w0, w1, w2 = w_sel[..., 0], w_sel[..., 1], w_sel[..., 2]
    wt = np.stack([w0, (w0 + w1 + w2) * 0.5, (w0 - w1 + w2) * 0.5, w2], axis=-1)
    # lhsT layout: [ci, j*3+kh, co]  (j-major so j0's taps lead)
    w_lhsT = np.ascontiguousarray(
        wt.transpose(0, 2, 4, 3, 1).reshape(B, C, 12, OC)).astype(ml_dtypes.bfloat16)

    # de-interleaved padded columns: xe[k]=xpad[2k], xo[k]=xpad[2k+1]
    xeo = np.zeros((B, C, 2, H, WE), np.float32)
    xeo[:, :, 0, :, 1:WE] = x[:, :, :, 1::2]    # xe: cols 1,3..55
    xeo[:, :, 1, :, 0:G] = x[:, :, :, 0::2]     # xo: cols 0,2..54
    # host Winograd input transform (for each core's first NHOST images)
    xe0 = xeo[:, :, 0, :, 0:G]
    xe1 = xeo[:, :, 0, :, 1:G + 1]
    xo0 = xeo[:, :, 1, :, 0:G]
    xo1 = xeo[:, :, 1, :, 1:G + 1]
    D = np.stack([xe0 - xe1, xo0 + xe1, xe1 - xo0, xo0 - xo1],
                 axis=2).astype(ml_dtypes.bfloat16)          # [B, C, 4, H, G]
    xeo = xeo.astype(ml_dtypes.bfloat16)

    nc = _get_program()
    in_maps = []
    for k in range(NCORES):
        s = k * IPC
        in_maps.append({
            "d0": D[s : s + NHOST],
            "x": xeo[s + NHOST : s + IPC],
            "w": w_lhsT[s : s + IPC],
        })
    res = run_bass_kernel_spmd(nc, in_maps, list(range(NCORES)), **run_kwargs)
    o = np.concatenate([res.results[k]["o"] for k in range(NCORES)], axis=0)
    o = o.astype(np.float32)                     # [B, OC, 2, H, G]
    out = np.empty((B, OC, H, W), np.float32)
    out[:, :, :, 0::2] = o[:, :, 0]
    out[:, :, :, 1::2] = o[:, :, 1]
    if run_kwargs:
        kernel.last_results = res
    return out
